# revision 26
# baseline (speedup 1.0000x reference)
"""Nearest-color-distance loss on 8 TRN2 NeuronCores.

loss = mean_i min_j ||x_i - p_j||_2,  x: (131072, 3), p: (128, 3).

Candidate-pruned kNN: the host kd-partitions all 131072 colors into
1024 chunks of exactly 128 spatially-close colors (median splits,
~0.1-side boxes) and, per chunk, selects the palette entries that can
possibly be the nearest neighbour of ANY point in the chunk's bbox
(mindist(j,box) <= min_k maxdist(k,box) -- an exact superset). On
this input that set has mean ~8, max ~24 entries, so each chunk ships
a fixed list of C=32 candidates (padded by repeating a real candidate,
idempotent under min; a chunk that ever overflowed 32 would be
computed on host and masked out -- does not happen for uniform data).

Device per core: 128 chunks x 32 candidates. d2 via K=5 fp16 packing
([x0,x1,x2,1,xn] vs [-2p0,-2p1,-2p2,pn,1], norms from fp16-ROUNDED
points so the error stays geometric ~1e-3). K=20 only occupies one
32-row strip of the PE array, so 4 groups run CONCURRENTLY in the 4
row-groups (tile_position=(32b,0)); each quad of groups fills one
PSUM bank [128, 512] (16 chunks x 32). DVE tensor_reduce(min) eats
two adjacent banks per op ([128,2,16,32] -> [128,32]). 8 banks = 8
quads, zero PSUM reuse. Inputs are [128, 1024] fp16 tensors (banded
so the DMA engages all partitions, ~0.8us each, split + overlapped).
Output [128,128] fp16 (minv[:, ck] = min-d2 of chunk ck); host does
sqrt/mean in f64.
"""

import sys

sys.path.insert(0, "/opt/trn_rl_repo")

import numpy as np

import concourse.bass as bass
import concourse.tile as tile
from concourse import bacc, mybir
from concourse.alu_op_type import AluOpType

N_CORES = 8
N = 131072
NPC = N // N_CORES  # 16384 colors per core
M = 128  # palette size
C = 20  # candidate budget per chunk
QW = 128 + 4 * C  # columns per quad block in xin (stationary + candidates)
XW = 8 * QW  # xin width
F16 = mybir.dt.float16
F32 = mybir.dt.float32


def build_nc():
    nc = bacc.Bacc(
        "TRN2",
        target_bir_lowering=False,
        debug=False,
        enable_asserts=False,
        num_devices=N_CORES,
    )
    xin_d = nc.dram_tensor("xin", [128, XW], F16, kind="ExternalInput").ap()
    minv_d = nc.dram_tensor("minv", [128, 128], F16, kind="ExternalOutput").ap()

    with tile.TileContext(nc) as tc:
        with (
            tc.tile_pool(name="sb", bufs=1) as sb,
            tc.tile_pool(name="pp", bufs=2, space=bass.MemorySpace.PSUM) as pp,
        ):
            # xin interleaves per-quad blocks: cols 256*Q..256*Q+128 hold
            # the stationary colors (xtb), +128..+256 the candidate
            # features (pmov). Finer first pieces so quad 0 starts ASAP;
            # split across three DMA queues for parallel issue/transfer.
            xin = sb.tile([128, XW], F16)
            minv = sb.tile([128, 128], F16)

            pieces = [
                (nc.sync, 0, 2 * QW),
                (nc.scalar, 4 * QW, 6 * QW),
                (nc.sync, 2 * QW, 4 * QW),
                (nc.scalar, 6 * QW, 8 * QW),
            ]
            for q, a, b in pieces:
                q.dma_start(xin[:, a:b], xin_d[:, a:b])

            def xtb(k, Q):
                return xin[32 * k : 32 * k + 20, QW * Q : QW * Q + 128]

            def pmov(k, Q):
                return xin[32 * k : 32 * k + 20, QW * Q + 128 : QW * (Q + 1)]

            # Mega-tile m = 4 banks; row-group k owns bank k, quad Q sits
            # at column offset 128*(Q%4) inside each bank, so the 4
            # concurrent matmuls of a quad always write 4 DIFFERENT banks
            # (same-bank concurrent writes deadlock the PE).
            W = 4 * C  # columns per quad per bank
            for m in range(2):
                ps = pp.tile([128, 2048], F32)
                for g in range(4):
                    Q = 4 * m + g
                    for k in range(4):
                        nc.tensor.matmul(
                            ps[:, 512 * k + W * g : 512 * k + W * (g + 1)],
                            xtb(k, Q),
                            pmov(k, Q),
                            start=True,
                            stop=True,
                            tile_position=(32 * k, 0),
                        )
                    if g % 2 == 1:
                        # quad-pair reduce: (g,c) merge into one uniform-
                        # stride axis inside each bank -> [p, k, 8, C]
                        B = Q // 2
                        v = ps[:].rearrange("p (k r) -> p k r", k=4)
                        vp = v[
                            :, :, W * (g - 1) : W * (g + 1)
                        ].rearrange("p k (a j) -> p k a j", j=C)
                        nc.vector.tensor_reduce(
                            minv[:, 32 * B : 32 * (B + 1)].rearrange(
                                "p (k a) -> p k a", a=8
                            ),
                            vp,
                            axis=mybir.AxisListType.X,
                            op=AluOpType.min,
                        )
                if m == 0:
                    nc.scalar.dma_start(minv_d[:, 0:64], minv[:, 0:64])
                else:
                    nc.scalar.dma_start(minv_d[:, 64:96], minv[:, 64:96])
            nc.sync.dma_start(minv_d[:, 96:128], minv[:, 96:128])

    nc.compile()
    return nc


def kd_order(x, leaf=128):
    """Order colors so each consecutive `leaf` block is a kd-tree leaf."""
    out = []

    def rec(ids):
        if len(ids) <= leaf:
            out.append(ids)
            return
        xs = x[ids]
        ax = int(np.argmax(xs.max(0) - xs.min(0)))
        half = (len(ids) // 2 // leaf) * leaf
        if half == 0:
            half = leaf
        part = np.argpartition(xs[:, ax], half)
        rec(ids[part[:half]])
        rec(ids[part[half:]])

    rec(np.arange(len(x)))
    return np.concatenate(out)


def prep_inputs(output_colors, target_palette):
    pal = np.asarray(target_palette, dtype=np.float32)
    mu = pal.mean(axis=0)
    ph = (pal - mu).astype(np.float16)  # rounded centered palette
    phf = ph.astype(np.float32)
    pn = (phf * phf).sum(axis=1).astype(np.float16)  # norms of rounded pts

    x = np.asarray(output_colors, dtype=np.float32)
    order = kd_order(x)
    xc = x[order] - mu
    xh = xc.astype(np.float16)
    xhf = xh.astype(np.float32)
    xn = (xhf * xhf).sum(axis=1).astype(np.float16)

    # per-chunk candidate selection (exact superset via bbox criterion)
    NCH = N // 128  # 1024 chunks
    ch = xc.reshape(NCH, 128, 3)
    lo = ch.min(1)[:, None, :]
    hi = ch.max(1)[:, None, :]
    pc = (phf)[None, :, :]  # centered palette f32
    mind = np.linalg.norm(np.clip(pc, lo, hi) - pc, axis=2)
    maxd = np.linalg.norm(np.maximum(np.abs(pc - lo), np.abs(pc - hi)), axis=2)
    thresh = maxd.min(1, keepdims=True)
    cand = mind <= thresh  # (NCH, 128)
    ncand = cand.sum(1)
    overflow = ncand > C  # host-fallback chunks (expected: none)
    idx = np.argsort(~cand, axis=1, kind="stable")[:, :C]  # (NCH, C)
    padmask = np.arange(C)[None, :] >= np.minimum(ncand, C)[:, None]
    idxp = np.where(padmask, idx[:, :1], idx)  # pad with first candidate

    # candidate features [NCH, C]: -2p, pn, 1
    cf = np.empty((NCH, 5, C), dtype=np.float16)
    cf[:, 0:3, :] = (-2.0 * ph)[idxp].transpose(0, 2, 1)
    cf[:, 3, :] = pn[idxp]
    cf[:, 4, :] = 1.0

    feats = np.empty((NPC, 5), dtype=np.float16)
    in_maps = []
    host_vals = []  # per core: (overflow_cols, host-computed sqrt-sums)
    for k in range(N_CORES):
        sl = slice(k * NPC, (k + 1) * NPC)
        xs = xh[sl]
        feats[:, 0:3] = xs
        feats[:, 3] = 1.0
        feats[:, 4] = xn[sl]
        arr = feats.reshape(128, 128, 5)  # [ck, i, r]
        xin = np.zeros((128, XW), dtype=np.float16)
        for ck in range(128):
            # device minv col = 32*B + 8*band + 4*(Q%2) + c must equal ck
            B, b, gl, c = ck // 32, (ck % 32) // 8, (ck % 8) // 4, ck % 4
            Q = 2 * B + gl
            rows = slice(32 * b + 5 * c, 32 * b + 5 * c + 5)
            xin[rows, QW * Q : QW * Q + 128] = arr[ck].T
            xin[rows, QW * Q + 128 + C * c : QW * Q + 128 + C * (c + 1)] = (
                cf[k * 128 + ck]
            )
        ovf = np.nonzero(overflow[k * 128 : (k + 1) * 128])[0]
        hsum = 0.0
        if len(ovf):
            for ck in ovf:
                xs128 = xc[sl][ck * 128 : (ck + 1) * 128]
                d2 = ((xs128[:, None, :] - phf[None, :, :]) ** 2).sum(2)
                hsum += np.sqrt(d2.min(1)).sum(dtype=np.float64)
        host_vals.append((ovf, hsum))
        in_maps.append({"xin": xin})
    return in_maps, host_vals


_NC_CACHE = {}


def get_nc():
    if "nc" not in _NC_CACHE:
        _NC_CACHE["nc"] = build_nc()
    return _NC_CACHE["nc"]


def kernel(output_colors=None, target_palette=None, _trace=False, **_):
    from concourse.bass_utils import run_bass_kernel_spmd

    nc = get_nc()
    in_maps, host_vals = prep_inputs(output_colors, target_palette)
    res = run_bass_kernel_spmd(
        nc, in_maps, core_ids=list(range(N_CORES)), trace=_trace
    )
    total = np.float64(0.0)
    for r, (ovf, hsum) in zip(res.results, host_vals):
        mv = np.maximum(r["minv"].astype(np.float64), 0.0)  # [i, ck]
        if len(ovf):
            keep = np.ones(128, dtype=bool)
            keep[ovf] = False
            total += np.sqrt(mv[:, keep]).sum() + hsum
        else:
            total += np.sqrt(mv).sum()
    out = np.array(total / N, dtype=np.float32)
    if _trace:
        kernel._last_results = res
    return out


if __name__ == "__main__":
    rng = np.random.default_rng(0)
    oc = rng.random((N, 3), dtype=np.float32)
    tp = rng.random((M, 3), dtype=np.float32)
    got = kernel(output_colors=oc, target_palette=tp)
    d = oc[:, None, :] - tp[None, :, :]
    want = np.sqrt((d * d).sum(-1)).min(1).mean(dtype=np.float64)
    print("got", got, "want", want, "rel", abs(got - want) / abs(want))


# revision 27
# speedup vs baseline: 1.0774x; 1.0774x over previous
"""Nearest-color-distance loss on 8 TRN2 NeuronCores.

loss = mean_i min_j ||x_i - p_j||_2,  x: (131072, 3), p: (128, 3).

Candidate-pruned kNN: the host kd-partitions all 131072 colors into
1024 chunks of exactly 128 spatially-close colors (median splits,
~0.1-side boxes) and, per chunk, selects the palette entries that can
possibly be the nearest neighbour of ANY point in the chunk's bbox
(mindist(j,box) <= min_k maxdist(k,box) -- an exact superset). On
this input that set has mean ~8, max ~24 entries, so each chunk ships
a fixed list of C=32 candidates (padded by repeating a real candidate,
idempotent under min; a chunk that ever overflowed 32 would be
computed on host and masked out -- does not happen for uniform data).

Device per core: 128 chunks x 32 candidates. d2 via K=5 fp16 packing
([x0,x1,x2,1,xn] vs [-2p0,-2p1,-2p2,pn,1], norms from fp16-ROUNDED
points so the error stays geometric ~1e-3). K=20 only occupies one
32-row strip of the PE array, so 4 groups run CONCURRENTLY in the 4
row-groups (tile_position=(32b,0)); each quad of groups fills one
PSUM bank [128, 512] (16 chunks x 32). DVE tensor_reduce(min) eats
two adjacent banks per op ([128,2,16,32] -> [128,32]). 8 banks = 8
quads, zero PSUM reuse. Inputs are [128, 1024] fp16 tensors (banded
so the DMA engages all partitions, ~0.8us each, split + overlapped).
Output [128,128] fp16 (minv[:, ck] = min-d2 of chunk ck); host does
sqrt/mean in f64.
"""

import sys

sys.path.insert(0, "/opt/trn_rl_repo")

import numpy as np

import concourse.bass as bass
import concourse.tile as tile
from concourse import bacc, mybir
from concourse.alu_op_type import AluOpType

N_CORES = 8
N = 131072
NPC = N // N_CORES  # 16384 colors per core
M = 128  # palette size
C = 24  # candidate budget per chunk
QW = 128 + 4 * C  # columns per quad block in xin (stationary + candidates)
XW = 8 * QW  # xin width
F16 = mybir.dt.float16
F32 = mybir.dt.float32


def build_nc():
    nc = bacc.Bacc(
        "TRN2",
        target_bir_lowering=False,
        debug=False,
        enable_asserts=False,
        num_devices=N_CORES,
    )
    xin_d = nc.dram_tensor("xin", [128, XW], F16, kind="ExternalInput").ap()
    minv_d = nc.dram_tensor("minv", [128, 128], F16, kind="ExternalOutput").ap()

    with tile.TileContext(nc) as tc:
        with (
            tc.tile_pool(name="sb", bufs=1) as sb,
            tc.tile_pool(name="pp", bufs=2, space=bass.MemorySpace.PSUM) as pp,
        ):
            # xin interleaves per-quad blocks: cols 256*Q..256*Q+128 hold
            # the stationary colors (xtb), +128..+256 the candidate
            # features (pmov). Finer first pieces so quad 0 starts ASAP;
            # split across three DMA queues for parallel issue/transfer.
            xin = sb.tile([128, XW], F16)
            minv = sb.tile([128, 128], F16)

            pieces = [
                (nc.sync, 0, QW),
                (nc.scalar, 3 * QW, 5 * QW),
                (nc.sync, QW, 3 * QW),
                (nc.scalar, 5 * QW, 8 * QW),
            ]
            for q, a, b in pieces:
                q.dma_start(xin[:, a:b], xin_d[:, a:b])

            def xtb(k, Q):
                return xin[32 * k : 32 * k + 20, QW * Q : QW * Q + 128]

            def pmov(k, Q):
                return xin[32 * k : 32 * k + 20, QW * Q + 128 : QW * (Q + 1)]

            # Mega-tile m = 4 banks; row-group k owns bank k, quad Q sits
            # at column offset 128*(Q%4) inside each bank, so the 4
            # concurrent matmuls of a quad always write 4 DIFFERENT banks
            # (same-bank concurrent writes deadlock the PE).
            W = 4 * C  # columns per quad per bank
            for m in range(2):
                ps = pp.tile([128, 2048], F32)
                for g in range(4):
                    Q = 4 * m + g
                    for k in range(4):
                        nc.tensor.matmul(
                            ps[:, 512 * k + W * g : 512 * k + W * (g + 1)],
                            xtb(k, Q),
                            pmov(k, Q),
                            start=True,
                            stop=True,
                            tile_position=(32 * k, 0),
                        )
                    if g % 2 == 1:
                        # quad-pair reduce: (g,c) merge into one uniform-
                        # stride axis inside each bank -> [p, k, 8, C]
                        B = Q // 2
                        v = ps[:].rearrange("p (k r) -> p k r", k=4)
                        vp = v[
                            :, :, W * (g - 1) : W * (g + 1)
                        ].rearrange("p k (a j) -> p k a j", j=C)
                        nc.vector.tensor_reduce(
                            minv[:, 32 * B : 32 * (B + 1)].rearrange(
                                "p (k a) -> p k a", a=8
                            ),
                            vp,
                            axis=mybir.AxisListType.X,
                            op=AluOpType.min,
                        )
                if m == 0:
                    nc.scalar.dma_start(minv_d[:, 0:64], minv[:, 0:64])
            nc.sync.dma_start(minv_d[:, 64:128], minv[:, 64:128])

    nc.compile()
    return nc


def kd_order(x, leaf=128):
    """Order colors so each consecutive `leaf` block is a kd-tree leaf."""
    out = []

    def rec(ids):
        if len(ids) <= leaf:
            out.append(ids)
            return
        xs = x[ids]
        ax = int(np.argmax(xs.max(0) - xs.min(0)))
        half = (len(ids) // 2 // leaf) * leaf
        if half == 0:
            half = leaf
        part = np.argpartition(xs[:, ax], half)
        rec(ids[part[:half]])
        rec(ids[part[half:]])

    rec(np.arange(len(x)))
    return np.concatenate(out)


def prep_inputs(output_colors, target_palette):
    pal = np.asarray(target_palette, dtype=np.float32)
    mu = pal.mean(axis=0)
    ph = (pal - mu).astype(np.float16)  # rounded centered palette
    phf = ph.astype(np.float32)
    pn = (phf * phf).sum(axis=1).astype(np.float16)  # norms of rounded pts

    x = np.asarray(output_colors, dtype=np.float32)
    order = kd_order(x)
    xc = x[order] - mu
    xh = xc.astype(np.float16)
    xhf = xh.astype(np.float32)
    xn = (xhf * xhf).sum(axis=1).astype(np.float16)

    # per-chunk candidate selection (exact superset via bbox criterion)
    NCH = N // 128  # 1024 chunks
    ch = xc.reshape(NCH, 128, 3)
    lo = ch.min(1)[:, None, :]
    hi = ch.max(1)[:, None, :]
    pc = (phf)[None, :, :]  # centered palette f32
    mind = np.linalg.norm(np.clip(pc, lo, hi) - pc, axis=2)
    maxd = np.linalg.norm(np.maximum(np.abs(pc - lo), np.abs(pc - hi)), axis=2)
    thresh = maxd.min(1, keepdims=True)
    cand = mind <= thresh  # (NCH, 128)
    ncand = cand.sum(1)
    overflow = ncand > C  # host-fallback chunks (expected: none)
    idx = np.argsort(~cand, axis=1, kind="stable")[:, :C]  # (NCH, C)
    padmask = np.arange(C)[None, :] >= np.minimum(ncand, C)[:, None]
    idxp = np.where(padmask, idx[:, :1], idx)  # pad with first candidate

    # candidate features [NCH, C]: -2p, pn, 1
    cf = np.empty((NCH, 5, C), dtype=np.float16)
    cf[:, 0:3, :] = (-2.0 * ph)[idxp].transpose(0, 2, 1)
    cf[:, 3, :] = pn[idxp]
    cf[:, 4, :] = 1.0

    feats = np.empty((NPC, 5), dtype=np.float16)
    in_maps = []
    host_vals = []  # per core: (overflow_cols, host-computed sqrt-sums)
    for k in range(N_CORES):
        sl = slice(k * NPC, (k + 1) * NPC)
        xs = xh[sl]
        feats[:, 0:3] = xs
        feats[:, 3] = 1.0
        feats[:, 4] = xn[sl]
        arr = feats.reshape(128, 128, 5)  # [ck, i, r]
        xin = np.zeros((128, XW), dtype=np.float16)
        for ck in range(128):
            # device minv col = 32*B + 8*band + 4*(Q%2) + c must equal ck
            B, b, gl, c = ck // 32, (ck % 32) // 8, (ck % 8) // 4, ck % 4
            Q = 2 * B + gl
            rows = slice(32 * b + 5 * c, 32 * b + 5 * c + 5)
            xin[rows, QW * Q : QW * Q + 128] = arr[ck].T
            xin[rows, QW * Q + 128 + C * c : QW * Q + 128 + C * (c + 1)] = (
                cf[k * 128 + ck]
            )
        ovf = np.nonzero(overflow[k * 128 : (k + 1) * 128])[0]
        hsum = 0.0
        if len(ovf):
            for ck in ovf:
                xs128 = xc[sl][ck * 128 : (ck + 1) * 128]
                d2 = ((xs128[:, None, :] - phf[None, :, :]) ** 2).sum(2)
                hsum += np.sqrt(d2.min(1)).sum(dtype=np.float64)
        host_vals.append((ovf, hsum))
        in_maps.append({"xin": xin})
    return in_maps, host_vals


_NC_CACHE = {}


def get_nc():
    if "nc" not in _NC_CACHE:
        _NC_CACHE["nc"] = build_nc()
    return _NC_CACHE["nc"]


def kernel(output_colors=None, target_palette=None, _trace=False, **_):
    from concourse.bass_utils import run_bass_kernel_spmd

    nc = get_nc()
    in_maps, host_vals = prep_inputs(output_colors, target_palette)
    res = run_bass_kernel_spmd(
        nc, in_maps, core_ids=list(range(N_CORES)), trace=_trace
    )
    total = np.float64(0.0)
    for r, (ovf, hsum) in zip(res.results, host_vals):
        mv = np.maximum(r["minv"].astype(np.float64), 0.0)  # [i, ck]
        if len(ovf):
            keep = np.ones(128, dtype=bool)
            keep[ovf] = False
            total += np.sqrt(mv[:, keep]).sum() + hsum
        else:
            total += np.sqrt(mv).sum()
    out = np.array(total / N, dtype=np.float32)
    if _trace:
        kernel._last_results = res
    return out


if __name__ == "__main__":
    rng = np.random.default_rng(0)
    oc = rng.random((N, 3), dtype=np.float32)
    tp = rng.random((M, 3), dtype=np.float32)
    got = kernel(output_colors=oc, target_palette=tp)
    d = oc[:, None, :] - tp[None, :, :]
    want = np.sqrt((d * d).sum(-1)).min(1).mean(dtype=np.float64)
    print("got", got, "want", want, "rel", abs(got - want) / abs(want))
